# revision 17
# baseline (speedup 1.0000x reference)
"""Trainium2 Bass kernel for the MoE distiller problem.

Math (per reference):
    gh     = relu(x @ Wg1 + bg1)                  [B, GH]
    gating = softmax(gh @ Wg2 + bg2)              [B, E]
    h_e    = relu(x @ W1[e] + b1[e])              [B, H]
    eo_e   = h_e @ W2[e] + b2[e]                  [B, D]
    w1n/w2n = softmax(gating[:, :3] / [:, 3:])    [B, 3]
    final  = sum_e gating[:,e] * eo_e
    t1     = sum_{e<3} w1n[:,e] * eo_e ; t2 analogous for e>=3

Strategy: data-parallel over 8 cores (2048 batch rows each). On each core:
  - L1 GEMMs run feature-major (weights stationary, x^T streamed) so the
    ReLU+bias evacuation has per-partition bias and lands on ScalarE.
  - L2 GEMMs run batch-major (h^T tiles stationary, W2 streamed) so
    expert_out / final / teachers come out in natural [B, ...] layout and
    the gating-weighted combines are per-partition-scalar DVE ops.
  - All matmul operands are fp16 (full PE rate, fp32 PSUM accumulation).
  - Batch is processed in blocks of 1024 so final/t1/t2 accumulators fit
    in SBUF; weights stream once per block.
"""

import numpy as np
import ml_dtypes

import concourse.bass as bass
import concourse.mybir as mybir
import concourse.tile as tile
from concourse import bacc
from concourse.bass_utils import run_bass_kernel_spmd

BF16 = mybir.dt.float16
F32 = mybir.dt.float32
AF = mybir.ActivationFunctionType
ALU = mybir.AluOpType

B, D, H, E, GH = 16384, 768, 1024, 6, 1024
NCORES = 8
BC = B // NCORES            # 2048 batch rows per core
BBLK = 1024                 # batch block (accumulator residency)
NBLK = BC // BBLK           # 2
NBT = BBLK // 128           # 8 batch tiles per block
KD = D // 128               # 6  k-tiles over D
KH = H // 128               # 8  k-tiles over H
MH = H // 128               # 8  m-tiles over H (feature-major L1 out)
EPT = E // 2                # 3  experts per teacher


def _bcast_ap(ap: bass.AP, parts: int = 128) -> bass.AP:
    """Partition-stride-0 AP: read the same free line into every partition."""
    free = [list(p) for p in ap.ap]
    return bass.AP(tensor=ap.tensor, offset=ap.offset, ap=[[0, parts]] + free)


def _build(zero_b2: bool, zero_bg2: bool) -> bass.Bass:
    nc = bacc.Bacc()

    xt = nc.dram_tensor("xt", [D, BC], BF16, kind="ExternalInput")
    w1 = nc.dram_tensor("w1", [E, D, H], BF16, kind="ExternalInput")
    w2 = nc.dram_tensor("w2", [E, H, D], BF16, kind="ExternalInput")
    wg1 = nc.dram_tensor("wg1", [D, GH], BF16, kind="ExternalInput")
    wg2 = nc.dram_tensor("wg2", [GH, E], BF16, kind="ExternalInput")
    # biases arrive host-pre-arranged partition-major for contiguous DMA
    b1 = nc.dram_tensor("b1", [128, E * MH], F32, kind="ExternalInput")
    bg1 = nc.dram_tensor("bg1", [128, MH], F32, kind="ExternalInput")
    b2 = nc.dram_tensor("b2", [E, D], F32, kind="ExternalInput")
    bg2 = nc.dram_tensor("bg2", [E], F32, kind="ExternalInput")

    eo = nc.dram_tensor("eo", [BC, E, D], F32, kind="ExternalOutput")
    gate = nc.dram_tensor("gate", [BC, E], F32, kind="ExternalOutput")
    fin = nc.dram_tensor("fin", [BC, D], F32, kind="ExternalOutput")
    t1 = nc.dram_tensor("t1", [BC, D], F32, kind="ExternalOutput")
    t2 = nc.dram_tensor("t2", [BC, D], F32, kind="ExternalOutput")

    with tile.TileContext(nc) as tc:
        with (
            tc.tile_pool(name="consts", bufs=1) as consts,
            tc.tile_pool(name="w1p", bufs=2) as w1p,
            tc.tile_pool(name="w2p", bufs=2) as w2p,
            tc.tile_pool(name="hp", bufs=2) as hp,
            tc.tile_pool(name="accp", bufs=1) as accp,
            tc.tile_pool(name="eop", bufs=3) as eop,
            tc.tile_pool(name="gp", bufs=2) as gp,
            tc.tile_pool(name="b2p", bufs=2) as b2p,
            tc.tile_pool(name="psh", bufs=2, space="PSUM") as psh,
            tc.tile_pool(name="pse", bufs=2, space="PSUM") as pse,
        ):
            # ---- constants resident for the whole kernel ----
            # Biases first (ScalarE's first ACTIVATE queues behind them),
            # then fine-grained x/W loads, first compute tile's slices
            # first, so the first matmul group starts as early as possible.
            xt_sb = consts.tile([128, KD, BC], BF16)
            wg1_sb = consts.tile([128, KD, GH], BF16)
            # pair the first compute tile's (wg1, xt) k-slices so the first
            # matmul group starts as soon as its k=0 slices land; SP
            # dispatches descriptors serially (~0.6us each), so the bulky
            # non-critical x columns go out on GpSimd's queue in parallel,
            # and the second block's x columns are loaded mid-expert-1.
            for k in range(KD):
                nc.sync.dma_start(
                    out=wg1_sb[:, k, :], in_=wg1[k * 128:(k + 1) * 128, :]
                )
                nc.sync.dma_start(
                    out=xt_sb[:, k, 0:512], in_=xt[k * 128:(k + 1) * 128, 0:512]
                )
            b1_sb = consts.tile([128, E, MH], F32)
            nc.sync.dma_start(out=b1_sb, in_=b1[:])
            bg1_sb = consts.tile([128, MH], F32)
            nc.sync.dma_start(out=bg1_sb, in_=bg1[:])
            for k in range(KD):
                nc.gpsimd.dma_start(
                    out=xt_sb[:, k, 512:BBLK],
                    in_=xt[k * 128:(k + 1) * 128, 512:BBLK],
                )
            ident = consts.tile([32, 32], F32)
            from concourse.masks import make_identity
            make_identity(nc, ident)
            wg2_sb = consts.tile([128, KH, E], BF16)
            nc.sync.dma_start(
                out=wg2_sb, in_=wg2[:].rearrange("(k p) e -> p k e", p=128)
            )
            bg2bc = None
            if not zero_bg2:
                bg2bc = consts.tile([128, E], F32)
                nc.gpsimd.dma_start(out=bg2bc, in_=_bcast_ap(bg2[:]))

            def l1_unit(w_sb, bias_col, h_sb, blk):
                """h_sb[:, m, :] = relu(w_sb.T @ xT_block + bias) feature-major."""
                for m in range(MH):
                    ps = psh.tile([128, BBLK], F32, tag="psh")
                    for n in range(BBLK // 512):
                        c0 = blk * BBLK + n * 512
                        for k in range(KD):
                            nc.tensor.matmul(
                                ps[:, n * 512:(n + 1) * 512],
                                lhsT=w_sb[:, k, m * 128:(m + 1) * 128],
                                rhs=xt_sb[:, k, c0:c0 + 512],
                                start=(k == 0),
                                stop=(k == KD - 1),
                            )
                    nc.scalar.activation(
                        out=h_sb[:, m, :], in_=ps[:],
                        func=AF.Relu, bias=bias_col(m), scale=1.0,
                    )

            for blk in range(NBLK):
                # ================= gating =================
                gh_sb = hp.tile([128, KH, BBLK], BF16, tag="h")
                l1_unit(wg1_sb, lambda m: bg1_sb[:, m:m + 1], gh_sb, blk)

                # gw: [0:6] gating probs, [6:12] renormalized teacher gates
                gw = gp.tile([128, NBT, 2 * E], F32, tag="gw")
                sc = gp.tile([128, NBT, 3 * E], F32, tag="scratch")
                sums = gp.tile([128, 2, NBT], F32, tag="sums")

                # gating L2 feature-major: logits^T [E, BBLK] with Wg2 as the
                # (tiny) stationary operand, then per-tile PE transpose into
                # batch-major — far fewer PE cycles than 8 tiny matmuls/tile
                ps_l = pse.tile([128, 1024], F32, tag="pse")
                for n2 in range(BBLK // 512):
                    for k in range(KH):
                        nc.tensor.matmul(
                            ps_l[0:E, n2 * 512:(n2 + 1) * 512],
                            lhsT=wg2_sb[:, k, :],
                            rhs=gh_sb[:, k, n2 * 512:(n2 + 1) * 512],
                            start=(k == 0),
                            stop=(k == KH - 1),
                        )
                lgT = gp.tile([E, BBLK], F32, tag="lgT")
                nc.scalar.copy(out=lgT, in_=ps_l[0:E, :])
                for bt in range(NBT):
                    ps_t = pse.tile([128, 1024], F32, tag="pse")
                    nc.tensor.transpose(
                        ps_t[:, 0:E],
                        lgT[:, bt * 128:(bt + 1) * 128],
                        ident[0:E, 0:E],
                    )
                    if zero_bg2:
                        nc.scalar.copy(out=sc[:, bt, 0:E], in_=ps_t[:, 0:E])
                    else:
                        nc.vector.scalar_tensor_tensor(
                            out=sc[:, bt, 0:E], in0=ps_t[:, 0:E], scalar=0.0,
                            in1=bg2bc, op0=ALU.bypass, op1=ALU.add,
                        )

                # softmax over E (logit magnitudes are O(1); no max-sub needed)
                nc.scalar.activation(
                    out=sc[:, :, E:2 * E], in_=sc[:, :, 0:E], func=AF.Exp
                )
                nc.vector.tensor_reduce(
                    out=sums[:, 0, :], in_=sc[:, :, E:2 * E],
                    axis=mybir.AxisListType.X, op=ALU.add,
                )
                nc.vector.reciprocal(out=sums[:, 1, :], in_=sums[:, 0, :])
                nc.vector.tensor_tensor(
                    gw[:, :, 0:E],
                    sc[:, :, E:2 * E],
                    sums[:, 1, :, None].to_broadcast([128, NBT, E]),
                    ALU.mult,
                )
                nc.sync.dma_start(
                    out=gate[blk * BBLK:(blk + 1) * BBLK, :]
                    .rearrange("(t p) e -> p t e", p=128),
                    in_=gw[:, :, 0:E],
                )
                # renormalized per-teacher gates: softmax over halves of gating
                nc.scalar.activation(
                    out=sc[:, :, 2 * E:3 * E], in_=gw[:, :, 0:E], func=AF.Exp
                )
                for hf in range(2):
                    lo = 2 * E + hf * EPT
                    nc.vector.tensor_reduce(
                        out=sums[:, 0, :], in_=sc[:, :, lo:lo + EPT],
                        axis=mybir.AxisListType.X, op=ALU.add,
                    )
                    nc.vector.reciprocal(out=sums[:, 1, :], in_=sums[:, 0, :])
                    nc.vector.tensor_tensor(
                        gw[:, :, E + hf * EPT:E + (hf + 1) * EPT],
                        sc[:, :, lo:lo + EPT],
                        sums[:, 1, :, None].to_broadcast([128, NBT, EPT]),
                        ALU.mult,
                    )

                # ================= experts =================
                accf = accp.tile([128, NBT, D], F32, tag="accf")
                acct = None
                for e in range(E):
                    if blk == 0 and e == 1:
                        # second block's x columns — needed ~250us from now
                        for k in range(KD):
                            nc.sync.dma_start(
                                out=xt_sb[:, k, BBLK:BC],
                                in_=xt[k * 128:(k + 1) * 128, BBLK:BC],
                            )
                    w1_sb = w1p.tile([128, KD, H], BF16, tag="w1")
                    for k in range(KD):
                        nc.sync.dma_start(
                            out=w1_sb[:, k, :],
                            in_=w1[e, k * 128:(k + 1) * 128, :],
                        )
                    h_sb = hp.tile([128, KH, BBLK], BF16, tag="h")
                    l1_unit(w1_sb, lambda m, e=e: b1_sb[:, e, m:m + 1], h_sb, blk)

                    w2_sb = w2p.tile([128, KH, D], BF16, tag="w2")
                    for k in range(KH):
                        nc.sync.dma_start(
                            out=w2_sb[:, k, :],
                            in_=w2[e, k * 128:(k + 1) * 128, :],
                        )
                    b2bc = None
                    if not zero_b2:
                        b2bc = b2p.tile([128, D], F32, tag="b2bc")
                        nc.gpsimd.dma_start(out=b2bc, in_=_bcast_ap(b2[e, :]))

                    if e % EPT == 0:
                        acct = accp.tile([128, NBT, D], F32, tag="acct")

                    for bt in range(NBT):
                        ps = pse.tile([128, 1024], F32, tag="pse")
                        for d0, dn in ((0, 512), (512, 256)):
                            for k in range(KH):
                                nc.tensor.matmul(
                                    ps[:, d0:d0 + dn],
                                    lhsT=h_sb[:, k, bt * 128:(bt + 1) * 128],
                                    rhs=w2_sb[:, k, d0:d0 + dn],
                                    start=(k == 0),
                                    stop=(k == KH - 1),
                                )
                        eo_sb = eop.tile([128, D], F32, tag="eo")
                        if zero_b2:
                            nc.scalar.copy(out=eo_sb, in_=ps[:, 0:D])
                        else:
                            nc.vector.scalar_tensor_tensor(
                                out=eo_sb, in0=ps[:, 0:D], scalar=0.0,
                                in1=b2bc, op0=ALU.bypass, op1=ALU.add,
                            )
                        r0 = blk * BBLK + bt * 128
                        nc.sync.dma_start(out=eo[r0:r0 + 128, e, :], in_=eo_sb)

                        gsc = gw[:, bt, e:e + 1]
                        if e == 0:
                            nc.vector.tensor_scalar_mul(
                                accf[:, bt, :], eo_sb, gsc
                            )
                        else:
                            nc.vector.scalar_tensor_tensor(
                                out=accf[:, bt, :], in0=eo_sb, scalar=gsc,
                                in1=accf[:, bt, :], op0=ALU.mult, op1=ALU.add,
                            )
                        tsc = gw[:, bt, E + e:E + e + 1]
                        if e % EPT == 0:
                            nc.vector.tensor_scalar_mul(
                                acct[:, bt, :], eo_sb, tsc
                            )
                        else:
                            nc.vector.scalar_tensor_tensor(
                                out=acct[:, bt, :], in0=eo_sb, scalar=tsc,
                                in1=acct[:, bt, :], op0=ALU.mult, op1=ALU.add,
                            )
                        # flush finished accumulators per batch-tile so the
                        # stores overlap remaining compute instead of piling
                        # up at the end of the block
                        if e == EPT - 1:
                            nc.sync.dma_start(
                                out=t1[r0:r0 + 128, :], in_=acct[:, bt, :]
                            )
                        elif e == E - 1:
                            nc.sync.dma_start(
                                out=t2[r0:r0 + 128, :], in_=acct[:, bt, :]
                            )
                            nc.sync.dma_start(
                                out=fin[r0:r0 + 128, :], in_=accf[:, bt, :]
                            )
    nc.compile()
    return nc


_cache: dict = {}


def _get_nc(zero_b2: bool, zero_bg2: bool) -> bass.Bass:
    key = (zero_b2, zero_bg2)
    if key not in _cache:
        _cache[key] = _build(zero_b2, zero_bg2)
    return _cache[key]


def _prep_inputs(x, W1, b1, W2, b2, Wg1, bg1, Wg2, bg2):
    bf = mybir.dt.np(BF16)
    x = np.asarray(x, dtype=np.float32)
    w1b = np.ascontiguousarray(np.asarray(W1, np.float32).astype(bf))
    w2b = np.ascontiguousarray(np.asarray(W2, np.float32).astype(bf))
    wg1b = np.ascontiguousarray(np.asarray(Wg1, np.float32).astype(bf))
    wg2b = np.ascontiguousarray(np.asarray(Wg2, np.float32).astype(bf))
    # partition-major bias layouts: [E, H] -> [128, E*MH], [GH] -> [128, MH]
    b1 = np.ascontiguousarray(
        np.asarray(b1, np.float32).reshape(E, MH, 128)
        .transpose(2, 0, 1).reshape(128, E * MH)
    )
    bg1 = np.ascontiguousarray(
        np.asarray(bg1, np.float32).reshape(MH, 128).T
    )
    b2 = np.ascontiguousarray(np.asarray(b2, np.float32))
    bg2 = np.ascontiguousarray(np.asarray(bg2, np.float32))
    in_maps = []
    for c in range(NCORES):
        xs = x[c * BC:(c + 1) * BC, :]
        xtb = np.ascontiguousarray(xs.T.astype(bf))
        in_maps.append({
            "xt": xtb, "w1": w1b, "w2": w2b, "wg1": wg1b, "wg2": wg2b,
            "b1": b1, "bg1": bg1, "b2": b2, "bg2": bg2,
        })
    return in_maps


def _run(inputs: dict, trace: bool = False):
    b2 = np.asarray(inputs["b2"], np.float32)
    bg2 = np.asarray(inputs["bg2"], np.float32)
    zero_b2 = not np.any(b2)
    zero_bg2 = not np.any(bg2)
    nc = _get_nc(zero_b2, zero_bg2)
    in_maps = _prep_inputs(**inputs)
    res = run_bass_kernel_spmd(
        nc, in_maps, core_ids=list(range(NCORES)), trace=trace
    )
    outs = res.results
    expert_out = np.concatenate([o["eo"] for o in outs], axis=0)
    gating = np.concatenate([o["gate"] for o in outs], axis=0)
    final = np.concatenate([o["fin"] for o in outs], axis=0)
    teacher1 = np.concatenate([o["t1"] for o in outs], axis=0)
    teacher2 = np.concatenate([o["t2"] for o in outs], axis=0)
    return (expert_out, gating, final, teacher1, teacher2), res


def kernel(**inputs):
    out, _ = _run(inputs, trace=False)
    return out


def kernel_profiled(**inputs):
    out, res = _run(inputs, trace=True)
    return out, res


# revision 19
# speedup vs baseline: 1.0123x; 1.0123x over previous
"""Trainium2 Bass kernel for the MoE distiller problem.

Math (per reference):
    gh     = relu(x @ Wg1 + bg1)                  [B, GH]
    gating = softmax(gh @ Wg2 + bg2)              [B, E]
    h_e    = relu(x @ W1[e] + b1[e])              [B, H]
    eo_e   = h_e @ W2[e] + b2[e]                  [B, D]
    w1n/w2n = softmax(gating[:, :3] / [:, 3:])    [B, 3]
    final  = sum_e gating[:,e] * eo_e
    t1     = sum_{e<3} w1n[:,e] * eo_e ; t2 analogous for e>=3

Strategy: data-parallel over 8 cores (2048 batch rows each). On each core:
  - L1 GEMMs run feature-major (weights stationary, x^T streamed) so the
    ReLU+bias evacuation has per-partition bias and lands on ScalarE.
  - L2 GEMMs run batch-major (h^T tiles stationary, W2 streamed) so
    expert_out / final / teachers come out in natural [B, ...] layout and
    the gating-weighted combines are per-partition-scalar DVE ops.
  - All matmul operands are fp16 (full PE rate, fp32 PSUM accumulation).
  - Batch is processed in blocks of 1024 so final/t1/t2 accumulators fit
    in SBUF; weights stream once per block.
"""

import numpy as np
import ml_dtypes

import concourse.bass as bass
import concourse.mybir as mybir
import concourse.tile as tile
from concourse import bacc
from concourse.bass_utils import run_bass_kernel_spmd

BF16 = mybir.dt.float16
F32 = mybir.dt.float32
AF = mybir.ActivationFunctionType
ALU = mybir.AluOpType

B, D, H, E, GH = 16384, 768, 1024, 6, 1024
NCORES = 8
BC = B // NCORES            # 2048 batch rows per core
BBLK = 1024                 # batch block (accumulator residency)
NBLK = BC // BBLK           # 2
NBT = BBLK // 128           # 8 batch tiles per block
KD = D // 128               # 6  k-tiles over D
KH = H // 128               # 8  k-tiles over H
MH = H // 128               # 8  m-tiles over H (feature-major L1 out)
EPT = E // 2                # 3  experts per teacher


def _bcast_ap(ap: bass.AP, parts: int = 128) -> bass.AP:
    """Partition-stride-0 AP: read the same free line into every partition."""
    free = [list(p) for p in ap.ap]
    return bass.AP(tensor=ap.tensor, offset=ap.offset, ap=[[0, parts]] + free)


def _build(zero_b2: bool, zero_bg2: bool) -> bass.Bass:
    nc = bacc.Bacc()

    xt = nc.dram_tensor("xt", [D, BC], BF16, kind="ExternalInput")
    w1 = nc.dram_tensor("w1", [E, D, H], BF16, kind="ExternalInput")
    w2 = nc.dram_tensor("w2", [E, H, D], BF16, kind="ExternalInput")
    wg1 = nc.dram_tensor("wg1", [D, GH], BF16, kind="ExternalInput")
    wg2 = nc.dram_tensor("wg2", [GH, E], BF16, kind="ExternalInput")
    # biases arrive host-pre-arranged partition-major for contiguous DMA
    b1 = nc.dram_tensor("b1", [128, E * MH], F32, kind="ExternalInput")
    bg1 = nc.dram_tensor("bg1", [128, MH], F32, kind="ExternalInput")
    b2 = nc.dram_tensor("b2", [E, D], F32, kind="ExternalInput")
    bg2 = nc.dram_tensor("bg2", [E], F32, kind="ExternalInput")

    eo = nc.dram_tensor("eo", [BC, E, D], F32, kind="ExternalOutput")
    gate = nc.dram_tensor("gate", [BC, E], F32, kind="ExternalOutput")
    fin = nc.dram_tensor("fin", [BC, D], F32, kind="ExternalOutput")
    t1 = nc.dram_tensor("t1", [BC, D], F32, kind="ExternalOutput")
    t2 = nc.dram_tensor("t2", [BC, D], F32, kind="ExternalOutput")

    with tile.TileContext(nc) as tc:
        with (
            tc.tile_pool(name="consts", bufs=1) as consts,
            tc.tile_pool(name="w1p", bufs=2) as w1p,
            tc.tile_pool(name="w2p", bufs=2) as w2p,
            tc.tile_pool(name="hp", bufs=2) as hp,
            tc.tile_pool(name="accp", bufs=1) as accp,
            tc.tile_pool(name="eop", bufs=3) as eop,
            tc.tile_pool(name="gp", bufs=2) as gp,
            tc.tile_pool(name="b2p", bufs=2) as b2p,
            tc.tile_pool(name="psh", bufs=2, space="PSUM") as psh,
            tc.tile_pool(name="pse", bufs=2, space="PSUM") as pse,
        ):
            # ---- constants resident for the whole kernel ----
            # Biases first (ScalarE's first ACTIVATE queues behind them),
            # then fine-grained x/W loads, first compute tile's slices
            # first, so the first matmul group starts as early as possible.
            xt_sb = consts.tile([128, KD, BC], BF16)
            wg1_sb = consts.tile([128, KD, GH], BF16)
            # pair the first compute tile's (wg1, xt) k-slices so the first
            # matmul group starts as soon as its k=0 slices land; SP
            # dispatches descriptors serially (~0.6us each), so the bulky
            # non-critical x columns go out on GpSimd's queue in parallel,
            # and the second block's x columns are loaded mid-expert-1.
            for k in range(KD):
                nc.sync.dma_start(
                    out=wg1_sb[:, k, :], in_=wg1[k * 128:(k + 1) * 128, :]
                )
                nc.sync.dma_start(
                    out=xt_sb[:, k, 0:512], in_=xt[k * 128:(k + 1) * 128, 0:512]
                )
            b1_sb = consts.tile([128, E, MH], F32)
            nc.sync.dma_start(out=b1_sb, in_=b1[:])
            bg1_sb = consts.tile([128, MH], F32)
            nc.sync.dma_start(out=bg1_sb, in_=bg1[:])
            for k in range(KD):
                nc.gpsimd.dma_start(
                    out=xt_sb[:, k, 512:BBLK],
                    in_=xt[k * 128:(k + 1) * 128, 512:BBLK],
                )

            wg2_sb = consts.tile([128, KH, E], BF16)
            nc.sync.dma_start(
                out=wg2_sb, in_=wg2[:].rearrange("(k p) e -> p k e", p=128)
            )
            bg2bc = None
            if not zero_bg2:
                bg2bc = consts.tile([128, E], F32)
                nc.gpsimd.dma_start(out=bg2bc, in_=_bcast_ap(bg2[:]))

            def l1_unit(w_sb, bias_col, h_sb, blk):
                """h_sb[:, m, :] = relu(w_sb.T @ xT_block + bias) feature-major."""
                for m in range(MH):
                    ps = psh.tile([128, BBLK], F32, tag="psh")
                    for n in range(BBLK // 512):
                        c0 = blk * BBLK + n * 512
                        for k in range(KD):
                            nc.tensor.matmul(
                                ps[:, n * 512:(n + 1) * 512],
                                lhsT=w_sb[:, k, m * 128:(m + 1) * 128],
                                rhs=xt_sb[:, k, c0:c0 + 512],
                                start=(k == 0),
                                stop=(k == KD - 1),
                            )
                    nc.scalar.activation(
                        out=h_sb[:, m, :], in_=ps[:],
                        func=AF.Relu, bias=bias_col(m), scale=1.0,
                    )

            for blk in range(NBLK):
                # ================= gating =================
                gh_sb = hp.tile([128, KH, BBLK], BF16, tag="h")
                l1_unit(wg1_sb, lambda m: bg1_sb[:, m:m + 1], gh_sb, blk)

                # gw: [0:6] gating probs, [6:12] renormalized teacher gates
                gw = gp.tile([128, NBT, 2 * E], F32, tag="gw")
                sc = gp.tile([128, NBT, 3 * E], F32, tag="scratch")
                sums = gp.tile([128, 2, NBT], F32, tag="sums")

                for bt in range(NBT):
                    ps = pse.tile([128, 1024], F32, tag="pse")
                    for k in range(KH):
                        nc.tensor.matmul(
                            ps[:, 0:E],
                            lhsT=gh_sb[:, k, bt * 128:(bt + 1) * 128],
                            rhs=wg2_sb[:, k, :],
                            start=(k == 0),
                            stop=(k == KH - 1),
                        )
                    if zero_bg2:
                        nc.scalar.copy(out=sc[:, bt, 0:E], in_=ps[:, 0:E])
                    else:
                        nc.vector.scalar_tensor_tensor(
                            out=sc[:, bt, 0:E], in0=ps[:, 0:E], scalar=0.0,
                            in1=bg2bc, op0=ALU.bypass, op1=ALU.add,
                        )

                # softmax over E (logit magnitudes are O(1); no max-sub needed)
                nc.scalar.activation(
                    out=sc[:, :, E:2 * E], in_=sc[:, :, 0:E], func=AF.Exp
                )
                nc.vector.tensor_reduce(
                    out=sums[:, 0, :], in_=sc[:, :, E:2 * E],
                    axis=mybir.AxisListType.X, op=ALU.add,
                )
                nc.vector.reciprocal(out=sums[:, 1, :], in_=sums[:, 0, :])
                nc.vector.tensor_tensor(
                    gw[:, :, 0:E],
                    sc[:, :, E:2 * E],
                    sums[:, 1, :, None].to_broadcast([128, NBT, E]),
                    ALU.mult,
                )
                nc.sync.dma_start(
                    out=gate[blk * BBLK:(blk + 1) * BBLK, :]
                    .rearrange("(t p) e -> p t e", p=128),
                    in_=gw[:, :, 0:E],
                )
                # renormalized per-teacher gates: softmax over halves of gating
                nc.scalar.activation(
                    out=sc[:, :, 2 * E:3 * E], in_=gw[:, :, 0:E], func=AF.Exp
                )
                for hf in range(2):
                    lo = 2 * E + hf * EPT
                    nc.vector.tensor_reduce(
                        out=sums[:, 0, :], in_=sc[:, :, lo:lo + EPT],
                        axis=mybir.AxisListType.X, op=ALU.add,
                    )
                    nc.vector.reciprocal(out=sums[:, 1, :], in_=sums[:, 0, :])
                    nc.vector.tensor_tensor(
                        gw[:, :, E + hf * EPT:E + (hf + 1) * EPT],
                        sc[:, :, lo:lo + EPT],
                        sums[:, 1, :, None].to_broadcast([128, NBT, EPT]),
                        ALU.mult,
                    )

                # ================= experts =================
                accf = accp.tile([128, NBT, D], F32, tag="accf")
                acct = None
                for e in range(E):
                    if blk == 0 and e == 1:
                        # second block's x columns — needed ~250us from now
                        for k in range(KD):
                            nc.sync.dma_start(
                                out=xt_sb[:, k, BBLK:BC],
                                in_=xt[k * 128:(k + 1) * 128, BBLK:BC],
                            )
                    w1_sb = w1p.tile([128, KD, H], BF16, tag="w1")
                    for k in range(KD):
                        nc.sync.dma_start(
                            out=w1_sb[:, k, :],
                            in_=w1[e, k * 128:(k + 1) * 128, :],
                        )
                    h_sb = hp.tile([128, KH, BBLK], BF16, tag="h")
                    l1_unit(w1_sb, lambda m, e=e: b1_sb[:, e, m:m + 1], h_sb, blk)

                    w2_sb = w2p.tile([128, KH, D], BF16, tag="w2")
                    for k in range(KH):
                        nc.sync.dma_start(
                            out=w2_sb[:, k, :],
                            in_=w2[e, k * 128:(k + 1) * 128, :],
                        )
                    b2bc = None
                    if not zero_b2:
                        b2bc = b2p.tile([128, D], F32, tag="b2bc")
                        nc.gpsimd.dma_start(out=b2bc, in_=_bcast_ap(b2[e, :]))

                    if e % EPT == 0:
                        acct = accp.tile([128, NBT, D], F32, tag="acct")

                    for bt in range(NBT):
                        ps = pse.tile([128, 1024], F32, tag="pse")
                        for d0, dn in ((0, 512), (512, 256)):
                            for k in range(KH):
                                nc.tensor.matmul(
                                    ps[:, d0:d0 + dn],
                                    lhsT=h_sb[:, k, bt * 128:(bt + 1) * 128],
                                    rhs=w2_sb[:, k, d0:d0 + dn],
                                    start=(k == 0),
                                    stop=(k == KH - 1),
                                )
                        eo_sb = eop.tile([128, D], F32, tag="eo")
                        if zero_b2:
                            nc.scalar.copy(out=eo_sb, in_=ps[:, 0:D])
                        else:
                            nc.vector.scalar_tensor_tensor(
                                out=eo_sb, in0=ps[:, 0:D], scalar=0.0,
                                in1=b2bc, op0=ALU.bypass, op1=ALU.add,
                            )
                        r0 = blk * BBLK + bt * 128
                        nc.sync.dma_start(out=eo[r0:r0 + 128, e, :], in_=eo_sb)

                        gsc = gw[:, bt, e:e + 1]
                        if e == 0:
                            nc.vector.tensor_scalar_mul(
                                accf[:, bt, :], eo_sb, gsc
                            )
                        else:
                            nc.vector.scalar_tensor_tensor(
                                out=accf[:, bt, :], in0=eo_sb, scalar=gsc,
                                in1=accf[:, bt, :], op0=ALU.mult, op1=ALU.add,
                            )
                        tsc = gw[:, bt, E + e:E + e + 1]
                        if e % EPT == 0:
                            nc.vector.tensor_scalar_mul(
                                acct[:, bt, :], eo_sb, tsc
                            )
                        else:
                            nc.vector.scalar_tensor_tensor(
                                out=acct[:, bt, :], in0=eo_sb, scalar=tsc,
                                in1=acct[:, bt, :], op0=ALU.mult, op1=ALU.add,
                            )
                        # flush finished accumulators per batch-tile so the
                        # stores overlap remaining compute instead of piling
                        # up at the end of the block
                        if e == EPT - 1:
                            nc.sync.dma_start(
                                out=t1[r0:r0 + 128, :], in_=acct[:, bt, :]
                            )
                        elif e == E - 1:
                            nc.sync.dma_start(
                                out=t2[r0:r0 + 128, :], in_=acct[:, bt, :]
                            )
                            nc.sync.dma_start(
                                out=fin[r0:r0 + 128, :], in_=accf[:, bt, :]
                            )
    nc.compile()
    return nc


_cache: dict = {}


def _get_nc(zero_b2: bool, zero_bg2: bool) -> bass.Bass:
    key = (zero_b2, zero_bg2)
    if key not in _cache:
        _cache[key] = _build(zero_b2, zero_bg2)
    return _cache[key]


def _prep_inputs(x, W1, b1, W2, b2, Wg1, bg1, Wg2, bg2):
    bf = mybir.dt.np(BF16)
    x = np.asarray(x, dtype=np.float32)
    w1b = np.ascontiguousarray(np.asarray(W1, np.float32).astype(bf))
    w2b = np.ascontiguousarray(np.asarray(W2, np.float32).astype(bf))
    wg1b = np.ascontiguousarray(np.asarray(Wg1, np.float32).astype(bf))
    wg2b = np.ascontiguousarray(np.asarray(Wg2, np.float32).astype(bf))
    # partition-major bias layouts: [E, H] -> [128, E*MH], [GH] -> [128, MH]
    b1 = np.ascontiguousarray(
        np.asarray(b1, np.float32).reshape(E, MH, 128)
        .transpose(2, 0, 1).reshape(128, E * MH)
    )
    bg1 = np.ascontiguousarray(
        np.asarray(bg1, np.float32).reshape(MH, 128).T
    )
    b2 = np.ascontiguousarray(np.asarray(b2, np.float32))
    bg2 = np.ascontiguousarray(np.asarray(bg2, np.float32))
    in_maps = []
    for c in range(NCORES):
        xs = x[c * BC:(c + 1) * BC, :]
        xtb = np.ascontiguousarray(xs.T.astype(bf))
        in_maps.append({
            "xt": xtb, "w1": w1b, "w2": w2b, "wg1": wg1b, "wg2": wg2b,
            "b1": b1, "bg1": bg1, "b2": b2, "bg2": bg2,
        })
    return in_maps


def _run(inputs: dict, trace: bool = False):
    b2 = np.asarray(inputs["b2"], np.float32)
    bg2 = np.asarray(inputs["bg2"], np.float32)
    zero_b2 = not np.any(b2)
    zero_bg2 = not np.any(bg2)
    nc = _get_nc(zero_b2, zero_bg2)
    in_maps = _prep_inputs(**inputs)
    res = run_bass_kernel_spmd(
        nc, in_maps, core_ids=list(range(NCORES)), trace=trace
    )
    outs = res.results
    expert_out = np.concatenate([o["eo"] for o in outs], axis=0)
    gating = np.concatenate([o["gate"] for o in outs], axis=0)
    final = np.concatenate([o["fin"] for o in outs], axis=0)
    teacher1 = np.concatenate([o["t1"] for o in outs], axis=0)
    teacher2 = np.concatenate([o["t2"] for o in outs], axis=0)
    return (expert_out, gating, final, teacher1, teacher2), res


def kernel(**inputs):
    out, _ = _run(inputs, trace=False)
    return out


def kernel_profiled(**inputs):
    out, res = _run(inputs, trace=True)
    return out, res


# revision 20
# speedup vs baseline: 1.0156x; 1.0032x over previous
"""Trainium2 Bass kernel for the MoE distiller problem.

Math (per reference):
    gh     = relu(x @ Wg1 + bg1)                  [B, GH]
    gating = softmax(gh @ Wg2 + bg2)              [B, E]
    h_e    = relu(x @ W1[e] + b1[e])              [B, H]
    eo_e   = h_e @ W2[e] + b2[e]                  [B, D]
    w1n/w2n = softmax(gating[:, :3] / [:, 3:])    [B, 3]
    final  = sum_e gating[:,e] * eo_e
    t1     = sum_{e<3} w1n[:,e] * eo_e ; t2 analogous for e>=3

Strategy: data-parallel over 8 cores (2048 batch rows each). On each core:
  - L1 GEMMs run feature-major (weights stationary, x^T streamed) so the
    ReLU+bias evacuation has per-partition bias and lands on ScalarE.
  - L2 GEMMs run batch-major (h^T tiles stationary, W2 streamed) so
    expert_out / final / teachers come out in natural [B, ...] layout and
    the gating-weighted combines are per-partition-scalar DVE ops.
  - All matmul operands are fp16 (full PE rate, fp32 PSUM accumulation).
  - Batch is processed in blocks of 1024 so final/t1/t2 accumulators fit
    in SBUF; weights stream once per block.
"""

import numpy as np
import ml_dtypes

import concourse.bass as bass
import concourse.mybir as mybir
import concourse.tile as tile
from concourse import bacc
from concourse.bass_utils import run_bass_kernel_spmd

BF16 = mybir.dt.float16
F32 = mybir.dt.float32
AF = mybir.ActivationFunctionType
ALU = mybir.AluOpType

B, D, H, E, GH = 16384, 768, 1024, 6, 1024
NCORES = 8
BC = B // NCORES            # 2048 batch rows per core
BBLK = 1024                 # batch block (accumulator residency)
NBLK = BC // BBLK           # 2
NBT = BBLK // 128           # 8 batch tiles per block
KD = D // 128               # 6  k-tiles over D
KH = H // 128               # 8  k-tiles over H
MH = H // 128               # 8  m-tiles over H (feature-major L1 out)
EPT = E // 2                # 3  experts per teacher


def _bcast_ap(ap: bass.AP, parts: int = 128) -> bass.AP:
    """Partition-stride-0 AP: read the same free line into every partition."""
    free = [list(p) for p in ap.ap]
    return bass.AP(tensor=ap.tensor, offset=ap.offset, ap=[[0, parts]] + free)


def _build(zero_b2: bool, zero_bg2: bool) -> bass.Bass:
    nc = bacc.Bacc()

    xt = nc.dram_tensor("xt", [D, BC], BF16, kind="ExternalInput")
    w1 = nc.dram_tensor("w1", [E, D, H], BF16, kind="ExternalInput")
    w2 = nc.dram_tensor("w2", [E, H, D], BF16, kind="ExternalInput")
    wg1 = nc.dram_tensor("wg1", [D, GH], BF16, kind="ExternalInput")
    wg2 = nc.dram_tensor("wg2", [GH, E], BF16, kind="ExternalInput")
    # biases arrive host-pre-arranged partition-major for contiguous DMA
    b1 = nc.dram_tensor("b1", [128, E * MH], F32, kind="ExternalInput")
    bg1 = nc.dram_tensor("bg1", [128, MH], F32, kind="ExternalInput")
    b2 = nc.dram_tensor("b2", [E, D], F32, kind="ExternalInput")
    bg2 = nc.dram_tensor("bg2", [E], F32, kind="ExternalInput")

    eo = nc.dram_tensor("eo", [BC, E, D], F32, kind="ExternalOutput")
    gate = nc.dram_tensor("gate", [BC, E], F32, kind="ExternalOutput")
    fin = nc.dram_tensor("fin", [BC, D], F32, kind="ExternalOutput")
    t1 = nc.dram_tensor("t1", [BC, D], F32, kind="ExternalOutput")
    t2 = nc.dram_tensor("t2", [BC, D], F32, kind="ExternalOutput")

    with tile.TileContext(nc) as tc:
        with (
            tc.tile_pool(name="consts", bufs=1) as consts,
            tc.tile_pool(name="w1p", bufs=2) as w1p,
            tc.tile_pool(name="w2p", bufs=2) as w2p,
            tc.tile_pool(name="hp", bufs=2) as hp,
            tc.tile_pool(name="accp", bufs=1) as accp,
            tc.tile_pool(name="eop", bufs=3) as eop,
            tc.tile_pool(name="gp", bufs=2) as gp,
            tc.tile_pool(name="b2p", bufs=2) as b2p,
            tc.tile_pool(name="psh", bufs=2, space="PSUM") as psh,
            tc.tile_pool(name="pse", bufs=2, space="PSUM") as pse,
        ):
            # ---- constants resident for the whole kernel ----
            # Biases first (ScalarE's first ACTIVATE queues behind them),
            # then fine-grained x/W loads, first compute tile's slices
            # first, so the first matmul group starts as early as possible.
            xt_sb = consts.tile([128, KD, BC], BF16)
            wg1_sb = consts.tile([128, KD, GH], BF16)
            # pair the first compute tile's (wg1, xt) k-slices so the first
            # matmul group starts as soon as its k=0 slices land; SP
            # dispatches descriptors serially (~0.6us each), so the bulky
            # non-critical x columns go out on GpSimd's queue in parallel,
            # and the second block's x columns are loaded mid-expert-1.
            for k in range(KD):
                nc.sync.dma_start(
                    out=wg1_sb[:, k, 0:512],
                    in_=wg1[k * 128:(k + 1) * 128, 0:512],
                )
                nc.sync.dma_start(
                    out=xt_sb[:, k, 0:512], in_=xt[k * 128:(k + 1) * 128, 0:512]
                )
            for k in range(KD):
                nc.sync.dma_start(
                    out=wg1_sb[:, k, 512:GH],
                    in_=wg1[k * 128:(k + 1) * 128, 512:GH],
                )
            b1_sb = consts.tile([128, E, MH], F32)
            nc.sync.dma_start(out=b1_sb, in_=b1[:])
            bg1_sb = consts.tile([128, MH], F32)
            nc.sync.dma_start(out=bg1_sb, in_=bg1[:])
            for k in range(KD):
                nc.gpsimd.dma_start(
                    out=xt_sb[:, k, 512:BBLK],
                    in_=xt[k * 128:(k + 1) * 128, 512:BBLK],
                )

            wg2_sb = consts.tile([128, KH, E], BF16)
            nc.sync.dma_start(
                out=wg2_sb, in_=wg2[:].rearrange("(k p) e -> p k e", p=128)
            )
            bg2bc = None
            if not zero_bg2:
                bg2bc = consts.tile([128, E], F32)
                nc.gpsimd.dma_start(out=bg2bc, in_=_bcast_ap(bg2[:]))

            def l1_unit(w_sb, bias_col, h_sb, blk):
                """h_sb[:, m, :] = relu(w_sb.T @ xT_block + bias) feature-major."""
                for m in range(MH):
                    ps = psh.tile([128, BBLK], F32, tag="psh")
                    for n in range(BBLK // 512):
                        c0 = blk * BBLK + n * 512
                        for k in range(KD):
                            nc.tensor.matmul(
                                ps[:, n * 512:(n + 1) * 512],
                                lhsT=w_sb[:, k, m * 128:(m + 1) * 128],
                                rhs=xt_sb[:, k, c0:c0 + 512],
                                start=(k == 0),
                                stop=(k == KD - 1),
                            )
                    nc.scalar.activation(
                        out=h_sb[:, m, :], in_=ps[:],
                        func=AF.Relu, bias=bias_col(m), scale=1.0,
                    )

            for blk in range(NBLK):
                # ================= gating =================
                gh_sb = hp.tile([128, KH, BBLK], BF16, tag="h")
                l1_unit(wg1_sb, lambda m: bg1_sb[:, m:m + 1], gh_sb, blk)

                # gw: [0:6] gating probs, [6:12] renormalized teacher gates
                gw = gp.tile([128, NBT, 2 * E], F32, tag="gw")
                sc = gp.tile([128, NBT, 3 * E], F32, tag="scratch")
                sums = gp.tile([128, 2, NBT], F32, tag="sums")

                for bt in range(NBT):
                    ps = pse.tile([128, 1024], F32, tag="pse")
                    for k in range(KH):
                        nc.tensor.matmul(
                            ps[:, 0:E],
                            lhsT=gh_sb[:, k, bt * 128:(bt + 1) * 128],
                            rhs=wg2_sb[:, k, :],
                            start=(k == 0),
                            stop=(k == KH - 1),
                        )
                    if zero_bg2:
                        nc.scalar.copy(out=sc[:, bt, 0:E], in_=ps[:, 0:E])
                    else:
                        nc.vector.scalar_tensor_tensor(
                            out=sc[:, bt, 0:E], in0=ps[:, 0:E], scalar=0.0,
                            in1=bg2bc, op0=ALU.bypass, op1=ALU.add,
                        )

                # softmax over E (logit magnitudes are O(1); no max-sub needed)
                nc.scalar.activation(
                    out=sc[:, :, E:2 * E], in_=sc[:, :, 0:E], func=AF.Exp
                )
                nc.vector.tensor_reduce(
                    out=sums[:, 0, :], in_=sc[:, :, E:2 * E],
                    axis=mybir.AxisListType.X, op=ALU.add,
                )
                nc.vector.reciprocal(out=sums[:, 1, :], in_=sums[:, 0, :])
                nc.vector.tensor_tensor(
                    gw[:, :, 0:E],
                    sc[:, :, E:2 * E],
                    sums[:, 1, :, None].to_broadcast([128, NBT, E]),
                    ALU.mult,
                )
                nc.sync.dma_start(
                    out=gate[blk * BBLK:(blk + 1) * BBLK, :]
                    .rearrange("(t p) e -> p t e", p=128),
                    in_=gw[:, :, 0:E],
                )
                # renormalized per-teacher gates: softmax over halves of gating
                nc.scalar.activation(
                    out=sc[:, :, 2 * E:3 * E], in_=gw[:, :, 0:E], func=AF.Exp
                )
                for hf in range(2):
                    lo = 2 * E + hf * EPT
                    nc.vector.tensor_reduce(
                        out=sums[:, 0, :], in_=sc[:, :, lo:lo + EPT],
                        axis=mybir.AxisListType.X, op=ALU.add,
                    )
                    nc.vector.reciprocal(out=sums[:, 1, :], in_=sums[:, 0, :])
                    nc.vector.tensor_tensor(
                        gw[:, :, E + hf * EPT:E + (hf + 1) * EPT],
                        sc[:, :, lo:lo + EPT],
                        sums[:, 1, :, None].to_broadcast([128, NBT, EPT]),
                        ALU.mult,
                    )

                # ================= experts =================
                accf = accp.tile([128, NBT, D], F32, tag="accf")
                acct = None
                for e in range(E):
                    if blk == 0 and e == 1:
                        # second block's x columns — needed ~250us from now
                        for k in range(KD):
                            nc.sync.dma_start(
                                out=xt_sb[:, k, BBLK:BC],
                                in_=xt[k * 128:(k + 1) * 128, BBLK:BC],
                            )
                    w1_sb = w1p.tile([128, KD, H], BF16, tag="w1")
                    for k in range(KD):
                        nc.sync.dma_start(
                            out=w1_sb[:, k, :],
                            in_=w1[e, k * 128:(k + 1) * 128, :],
                        )
                    h_sb = hp.tile([128, KH, BBLK], BF16, tag="h")
                    l1_unit(w1_sb, lambda m, e=e: b1_sb[:, e, m:m + 1], h_sb, blk)

                    w2_sb = w2p.tile([128, KH, D], BF16, tag="w2")
                    for k in range(KH):
                        nc.sync.dma_start(
                            out=w2_sb[:, k, :],
                            in_=w2[e, k * 128:(k + 1) * 128, :],
                        )
                    b2bc = None
                    if not zero_b2:
                        b2bc = b2p.tile([128, D], F32, tag="b2bc")
                        nc.gpsimd.dma_start(out=b2bc, in_=_bcast_ap(b2[e, :]))

                    if e % EPT == 0:
                        acct = accp.tile([128, NBT, D], F32, tag="acct")

                    for bt in range(NBT):
                        ps = pse.tile([128, 1024], F32, tag="pse")
                        for d0, dn in ((0, 512), (512, 256)):
                            for k in range(KH):
                                nc.tensor.matmul(
                                    ps[:, d0:d0 + dn],
                                    lhsT=h_sb[:, k, bt * 128:(bt + 1) * 128],
                                    rhs=w2_sb[:, k, d0:d0 + dn],
                                    start=(k == 0),
                                    stop=(k == KH - 1),
                                )
                        eo_sb = eop.tile([128, D], F32, tag="eo")
                        if zero_b2:
                            nc.scalar.copy(out=eo_sb, in_=ps[:, 0:D])
                        else:
                            nc.vector.scalar_tensor_tensor(
                                out=eo_sb, in0=ps[:, 0:D], scalar=0.0,
                                in1=b2bc, op0=ALU.bypass, op1=ALU.add,
                            )
                        r0 = blk * BBLK + bt * 128
                        nc.sync.dma_start(out=eo[r0:r0 + 128, e, :], in_=eo_sb)

                        gsc = gw[:, bt, e:e + 1]
                        if e == 0:
                            nc.vector.tensor_scalar_mul(
                                accf[:, bt, :], eo_sb, gsc
                            )
                        else:
                            nc.vector.scalar_tensor_tensor(
                                out=accf[:, bt, :], in0=eo_sb, scalar=gsc,
                                in1=accf[:, bt, :], op0=ALU.mult, op1=ALU.add,
                            )
                        tsc = gw[:, bt, E + e:E + e + 1]
                        if e % EPT == 0:
                            nc.vector.tensor_scalar_mul(
                                acct[:, bt, :], eo_sb, tsc
                            )
                        else:
                            nc.vector.scalar_tensor_tensor(
                                out=acct[:, bt, :], in0=eo_sb, scalar=tsc,
                                in1=acct[:, bt, :], op0=ALU.mult, op1=ALU.add,
                            )
                        # flush finished accumulators per batch-tile so the
                        # stores overlap remaining compute instead of piling
                        # up at the end of the block
                        if e == EPT - 1:
                            nc.sync.dma_start(
                                out=t1[r0:r0 + 128, :], in_=acct[:, bt, :]
                            )
                        elif e == E - 1:
                            nc.sync.dma_start(
                                out=t2[r0:r0 + 128, :], in_=acct[:, bt, :]
                            )
                            nc.sync.dma_start(
                                out=fin[r0:r0 + 128, :], in_=accf[:, bt, :]
                            )
    nc.compile()
    return nc


_cache: dict = {}


def _get_nc(zero_b2: bool, zero_bg2: bool) -> bass.Bass:
    key = (zero_b2, zero_bg2)
    if key not in _cache:
        _cache[key] = _build(zero_b2, zero_bg2)
    return _cache[key]


def _prep_inputs(x, W1, b1, W2, b2, Wg1, bg1, Wg2, bg2):
    bf = mybir.dt.np(BF16)
    x = np.asarray(x, dtype=np.float32)
    w1b = np.ascontiguousarray(np.asarray(W1, np.float32).astype(bf))
    w2b = np.ascontiguousarray(np.asarray(W2, np.float32).astype(bf))
    wg1b = np.ascontiguousarray(np.asarray(Wg1, np.float32).astype(bf))
    wg2b = np.ascontiguousarray(np.asarray(Wg2, np.float32).astype(bf))
    # partition-major bias layouts: [E, H] -> [128, E*MH], [GH] -> [128, MH]
    b1 = np.ascontiguousarray(
        np.asarray(b1, np.float32).reshape(E, MH, 128)
        .transpose(2, 0, 1).reshape(128, E * MH)
    )
    bg1 = np.ascontiguousarray(
        np.asarray(bg1, np.float32).reshape(MH, 128).T
    )
    b2 = np.ascontiguousarray(np.asarray(b2, np.float32))
    bg2 = np.ascontiguousarray(np.asarray(bg2, np.float32))
    in_maps = []
    for c in range(NCORES):
        xs = x[c * BC:(c + 1) * BC, :]
        xtb = np.ascontiguousarray(xs.T.astype(bf))
        in_maps.append({
            "xt": xtb, "w1": w1b, "w2": w2b, "wg1": wg1b, "wg2": wg2b,
            "b1": b1, "bg1": bg1, "b2": b2, "bg2": bg2,
        })
    return in_maps


def _run(inputs: dict, trace: bool = False):
    b2 = np.asarray(inputs["b2"], np.float32)
    bg2 = np.asarray(inputs["bg2"], np.float32)
    zero_b2 = not np.any(b2)
    zero_bg2 = not np.any(bg2)
    nc = _get_nc(zero_b2, zero_bg2)
    in_maps = _prep_inputs(**inputs)
    res = run_bass_kernel_spmd(
        nc, in_maps, core_ids=list(range(NCORES)), trace=trace
    )
    outs = res.results
    expert_out = np.concatenate([o["eo"] for o in outs], axis=0)
    gating = np.concatenate([o["gate"] for o in outs], axis=0)
    final = np.concatenate([o["fin"] for o in outs], axis=0)
    teacher1 = np.concatenate([o["t1"] for o in outs], axis=0)
    teacher2 = np.concatenate([o["t2"] for o in outs], axis=0)
    return (expert_out, gating, final, teacher1, teacher2), res


def kernel(**inputs):
    out, _ = _run(inputs, trace=False)
    return out


def kernel_profiled(**inputs):
    out, res = _run(inputs, trace=True)
    return out, res
